# revision 42
# baseline (speedup 1.0000x reference)
"""Causal self-attention on 8 trn2 NeuronCores.

Sharding: data-parallel over batch (2) x tensor-parallel over heads (4/core).
Core c handles batch c//4, heads (c%4)*4 .. (c%4)*4+4.  Each core computes
QKV projection for its heads, causal attention, and a partial c_proj
(y_local @ w_proj[local rows]); the host sums the 4 partials per batch
(b_proj is folded in as b_proj/4 on every core).

Device kernel notes (v3):
- Matmul inputs are bf16 (host-converted); accumulation stays fp32 in PSUM.
- Attention uses the S^T = K Q^T orientation so the softmax reduction is a
  matmul: V is augmented with a ones column (col 64), so A@V also yields the
  softmax denominator in psum row 64.
- exp runs without max-subtraction (scores bounded for this problem family).
- Per kt tile, both heads' scores land in one 2-bank PSUM tile [128,1024]
  so one ACTIVATE covers both heads: 80 exp calls instead of 160 (per-call
  ACT overhead + per-call semaphores dominated the scalar queue at 160).
- The per-(qg,p) kt loop is software-pipelined with lag 2 (emit scores(kt),
  exp(kt), then AV(kt-2)): engine queues are strict FIFO, so without the
  lag the PE queue head blocks on the exp of the same tile and the PE goes
  idle (and HAM-cold) for ~2us per tile.
- AV psum is evacuated to yt (unnormalized) immediately so the next head
  pair can reuse the av psum tile; the 1/denominator scale is applied
  in-place on yt off the critical path.  c_proj of query group qg is
  emitted after the projections of qg+1 for the same reason.
- Inputs arrive in 10 large DMA descriptors split over both HWDGE queues
  (sync + scalar); descriptor issue costs ~600ns each and serialized the
  first 34us when done one tile at a time.  x^T loads in 4 column-block
  descriptors so query group g's projections start once its block lands.
"""

import os
import sys

for p in ("/root/.axon_site", "/root/.axon_site/_ro/trn_rl_repo", "/root/.axon_site/_ro/pypackages", "/opt/trn_rl_repo"):
    if os.path.isdir(p) and p not in sys.path:
        sys.path.append(p)

import ml_dtypes
import numpy as np

import concourse.bacc as bacc
import concourse.mybir as mybir
import concourse.tile as tile
from concourse.bass_utils import run_bass_kernel_spmd

F32 = mybir.dt.float32
BF16 = mybir.dt.bfloat16
Exp = mybir.ActivationFunctionType.Exp
MULT = mybir.AluOpType.mult
ADD = mybir.AluOpType.add
BF = ml_dtypes.bfloat16

T = 2048            # sequence length (per batch)
C = 1024            # embedding dim
NHL = 4             # heads per core
HD = 64             # head dim
FL = NHL * HD       # local features (256)
CK = C // 128       # contraction chunks (8)
W3 = 3 * FL         # packed weight row: [wk | wq | wv] (768)
NQG = T // 512      # query groups of 512 (4)
NTT = T // 128      # token tiles of 128 (16)

_CACHE = {}
LAST_RESULTS = None


def _build():
    nc = bacc.Bacc("TRN2", target_bir_lowering=False, debug=False, num_devices=8)

    # x^T arrives block-major ([p, g, ck, 512] image) so each token-block DMA
    # is one fully contiguous descriptor
    x_img = nc.dram_tensor("x_img", [128, CK * T], BF16, kind="ExternalInput").ap()
    # weights arrive pre-arranged as the exact SBUF image (contiguous DMA:
    # strided descriptors cost ~1.7-3us each to issue on the HWDGE queue)
    wk_img = nc.dram_tensor("wk_img", [128, CK * 2 * 128], BF16, kind="ExternalInput").ap()
    wqv_img = nc.dram_tensor("wqv_img", [128, CK * 4 * 128], BF16, kind="ExternalInput").ap()
    wp_img = nc.dram_tensor("wp_img", [128, 2 * C], BF16, kind="ExternalInput").ap()
    bkq = nc.dram_tensor("bkq", [128, 4], F32, kind="ExternalInput").ap()
    bv = nc.dram_tensor("bv", [1, FL], F32, kind="ExternalInput").ap()
    tril2 = nc.dram_tensor("tril2", [128, 256], BF16, kind="ExternalInput").ap()
    vones = nc.dram_tensor("vones", [128, NTT * NHL], BF16, kind="ExternalInput").ap()
    out = nc.dram_tensor("out", [T, C], F32, kind="ExternalOutput").ap()

    with tile.TileContext(nc) as tc:
        with (
            tc.tile_pool(name="persist", bufs=1) as pp,
            tc.tile_pool(name="xpool", bufs=1) as xp,
            tc.tile_pool(name="attp", bufs=4) as ap_,
            tc.tile_pool(name="smallp", bufs=2) as sp,
            tc.tile_pool(name="outp", bufs=2) as op_,
            tc.tile_pool(name="proj_ps", bufs=2, space="PSUM") as pps,
            tc.tile_pool(name="stg_ps", bufs=2, space="PSUM") as sps,
            tc.tile_pool(name="av_ps", bufs=1, space="PSUM") as avps,
        ):
            # ---- persistent SBUF tensors ----
            xt_sb = xp.tile([128, CK * T], BF16)          # 8 chunks of x^T [128, 2048]
            wk_sb = pp.tile([128, CK * 256], BF16, tag="wk")   # 8 chunks of wk [128,256]
            wqv_sb = pp.tile([128, CK * 512], BF16, tag="wqv")  # 8 chunks of [wq|wv]
            qt_sb = [pp.tile([128, T], BF16, tag=f"qt{p}", name=f"qt{p}") for p in range(2)]
            kt_sb = [pp.tile([128, T], BF16, tag=f"kt{p}", name=f"kt{p}") for p in range(2)]
            v_sb = pp.tile([128, NTT * NHL * (HD + 1)], BF16, tag="v")  # per tile: 4x65
            yt_sb = [pp.tile([128, T], BF16, tag=f"yt{p}", name=f"yt{p}") for p in range(2)]
            wp_sb = pp.tile([128, 2 * C], BF16)
            bkq_sb = pp.tile([128, 4], F32, tag="bkq")
            bv_bc = pp.tile([128, FL], F32, tag="bvbc")
            tril_sb = pp.tile([128, 256], BF16, tag="tril")
            bv_row = pp.tile([1, FL], F32, tag="bvrow")
            vones_sb = pp.tile([128, NTT * NHL], BF16, tag="vones")

            # ---- input DMAs: few large contiguous descriptors on both HWDGE
            # queues; x arrives per token-block so qg0 can start early ----
            XB = CK * 512  # columns per token block in the block-major image
            for g in range(NQG):
                nc.sync.dma_start(out=xt_sb[:, g * XB:(g + 1) * XB],
                                  in_=x_img[:, g * XB:(g + 1) * XB])

            def xsl(ck, t0, n):
                """xt_sb slice for contraction chunk ck, tokens [t0, t0+n)."""
                g, j = divmod(t0, 512)
                c0 = g * XB + ck * 512 + j
                return xt_sb[:, c0:c0 + n]
            # contiguous weight images; wk first so K-proj starts after 0.5MB
            nc.scalar.dma_start(out=wk_sb[:], in_=wk_img)
            nc.scalar.dma_start(out=bkq_sb[:], in_=bkq)
            nc.scalar.dma_start(out=wqv_sb[:], in_=wqv_img)
            nc.scalar.dma_start(out=bv_row[:], in_=bv)
            nc.scalar.dma_start(out=vones_sb[:], in_=vones)
            nc.scalar.dma_start(out=tril_sb[:], in_=tril2)
            nc.scalar.dma_start(out=wp_sb[:], in_=wp_img)
            nc.gpsimd.partition_broadcast(bv_bc[:], bv_row[:])
            # ones column of the augmented V (col 64 of each head block)
            v_ones = v_sb[:].rearrange("p (n c) -> p n c", c=HD + 1)[:, :, HD]
            nc.vector.tensor_copy(v_ones, vones_sb[:])

            def v_tile(tt):
                return v_sb[:, tt * NHL * (HD + 1):(tt + 1) * NHL * (HD + 1)]

            def emit_proj(qg):
                """K^T, Q^T, V projections for query group qg."""
                qs = qg * 512
                for sel in range(2):  # 0: K, 1: Q
                    for p in range(2):
                        ps = pps.tile([128, 512], F32, tag="proj", name="proj_ps")
                        for ck in range(CK):
                            if sel == 0:
                                w = wk_sb[:, ck * 256 + p * 128: ck * 256 + (p + 1) * 128]
                            else:
                                w = wqv_sb[:, ck * 512 + p * 128: ck * 512 + (p + 1) * 128]
                            nc.tensor.matmul(
                                ps[:], w, xsl(ck, qs, 512),
                                start=(ck == 0), stop=(ck == CK - 1))
                        dst = (kt_sb, qt_sb)[sel]
                        bcol = (0, 2)[sel]
                        nc.vector.tensor_scalar_add(dst[p][:, qs:qs + 512], ps[:],
                                                    bkq_sb[:, bcol + p:bcol + p + 1])
                for tt in range(4 * qg, 4 * qg + 4):
                    ps = pps.tile([128, 512], F32, tag="proj", name="proj_ps")
                    for ck in range(CK):
                        nc.tensor.matmul(
                            ps[:, 0:FL],
                            xsl(ck, tt * 128, 128),
                            wqv_sb[:, ck * 512 + 256:(ck + 1) * 512],
                            start=(ck == 0), stop=(ck == CK - 1))
                    vdst = v_tile(tt).rearrange("p (n c) -> p n c", c=HD + 1)[:, :, 0:HD]
                    nc.vector.tensor_tensor(vdst, ps[:, 0:FL].rearrange("p (n c) -> p n c", c=HD),
                                            bv_bc[:].rearrange("p (n c) -> p n c", c=HD), ADD)

            def emit_attention(qg, p, fills=(), last=False):
                """Causal attention for query group qg, head pair p (lag-2 pipeline).

                fills: thunks of independent PE work spread through the kt loop
                to fill the bubbles of ACT-paced stretches (e.g. c_proj tiles
                during qg3, whose attention is exp-limited)."""
                qs = qg * 512
                K = 4 * qg + 4
                atts = [None] * K
                fill_at = {}
                if fills:
                    step = max(1, (K - 3) // max(1, len(fills) - 1)) if len(fills) > 1 else 1
                    for i, f in enumerate(fills):
                        fill_at[min(K - 1, 2 + i * step)] = f

                def emit_scores(kt):
                    d = max(0, (kt - 4 * qg) * 128)
                    stg = sps.tile([128, 1024], F32, tag="stg", name="stg_ps")
                    att = ap_.tile([128, 1024], BF16, tag="att", name="att")
                    atts[kt] = att
                    for h in range(2):
                        nc.tensor.matmul(
                            stg[:, h * 512 + d:(h + 1) * 512],
                            kt_sb[p][h * 64:(h + 1) * 64, kt * 128:(kt + 1) * 128],
                            qt_sb[p][h * 64:(h + 1) * 64, qs + d: qs + 512],
                            start=True, stop=True)
                    # one exp for both heads; on diagonal tiles skip the columns
                    # below the causal offset d via a strided view
                    if d:
                        nc.scalar.activation(
                            att[:].rearrange("q (h j) -> q h j", h=2)[:, :, d:512],
                            stg[:].rearrange("q (h j) -> q h j", h=2)[:, :, d:512],
                            Exp, scale=0.125)
                    else:
                        nc.scalar.activation(att[:], stg[:], Exp, scale=0.125)
                    if kt >= 4 * qg:
                        # in-place causal mask, both heads in one strided op
                        a = att[:].rearrange("q (h j) -> q h j", h=2)[:, :, d:d + 128]
                        m = tril_sb[:].rearrange("q (h j) -> q h j", h=2)
                        nc.vector.tensor_tensor(a, a, m, MULT)

                def emit_av(kt):
                    d = max(0, (kt - 4 * qg) * 128)
                    for h in range(2):
                        nc.tensor.matmul(
                            av[:, h * 512 + d:(h + 1) * 512],
                            v_tile(kt)[:, (2 * p + h) * (HD + 1):(2 * p + h + 1) * (HD + 1)],
                            atts[kt][:, h * 512 + d:(h + 1) * 512],
                            start=(kt == 0), stop=(kt == K - 1))

                # av: h0 in bank cols 0:512, h1 in 512:1024; row 64 = denominator
                av = avps.tile([65, 1024], F32, tag="av", name="av_ps")
                for kt in range(K):
                    emit_scores(kt)
                    if kt in fill_at:
                        fill_at[kt]()
                    if kt >= 2:
                        emit_av(kt - 2)
                emit_av(K - 2)
                emit_av(K - 1)
                # normalization chain; custom-DVE reciprocal misreads PSUM at
                # partition offset 64 on HW, and PSUM reads must start at an
                # aligned partition: stage the denominator row through SBUF.
                # The 1/denom partition-broadcast runs on gpsimd: it is idle,
                # so its queue blocking on the recip stalls nothing else (a PE
                # matmul broadcast measured 720ns each plus queue stalls).
                dsb = sp.tile([1, 1024], F32, tag="dsb", name="dsb")
                dinv = sp.tile([1, 1024], F32, tag="dinv", name="dinv")
                bc = sp.tile([128, 1024], F32, tag="bc", name="bc")
                if last:
                    # nothing reuses av: skip the early evacuation, and split
                    # the denominator copy across DVE+ACT to shorten the
                    # serial chain (ACT is idle here, DVE is the bottleneck)
                    nc.vector.tensor_copy(dsb[0:1, 0:512], av[64:65, 0:512])
                    nc.scalar.copy(dsb[0:1, 512:1024], av[64:65, 512:1024])
                    nc.vector.reciprocal_approx_fast(out=dinv[:], in_=dsb[:])
                    nc.gpsimd.partition_broadcast(bc[:], dinv[:])
                    for h in range(2):
                        nc.vector.tensor_tensor(
                            yt_sb[p][h * 64:(h + 1) * 64, qs:qs + 512],
                            av[0:64, h * 512:(h + 1) * 512],
                            bc[0:64, h * 512:(h + 1) * 512], MULT)
                else:
                    # evacuate av quickly (unnormalized) so the next head pair
                    # can reuse the av psum banks; scale yt in-place later
                    nc.vector.tensor_copy(dsb[:], av[64:65, :])
                    nc.vector.reciprocal_approx_fast(out=dinv[:], in_=dsb[:])
                    for h in range(2):
                        nc.vector.tensor_copy(yt_sb[p][h * 64:(h + 1) * 64, qs:qs + 512],
                                              av[0:64, h * 512:(h + 1) * 512])
                    nc.gpsimd.partition_broadcast(bc[:], dinv[:])
                    for h in range(2):
                        y = yt_sb[p][h * 64:(h + 1) * 64, qs:qs + 512]
                        nc.vector.tensor_tensor(
                            y, y, bc[h * 64:(h + 1) * 64, h * 512:(h + 1) * 512], MULT)

            def emit_cproj_tt(tt, on_scalar=False):
                """c_proj partial + output DMA for one 128-token tile.

                b_proj is added by the host after the partial sum, so the psum
                evacuation is a plain copy and can run on ACT (on_scalar=True,
                for the tail where ACT is idle but DVE is the bottleneck)."""
                ob = op_.tile([128, C], F32, tag="ob", name="ob")
                for ng in range(2):
                    ps = pps.tile([128, 512], F32, tag="proj", name="proj_ps")
                    for f in range(2):
                        nc.tensor.matmul(
                            ps[:],
                            yt_sb[f][:, tt * 128:(tt + 1) * 128],
                            wp_sb[:, f * C + ng * 512: f * C + ng * 512 + 512],
                            start=(f == 0), stop=(f == 1))
                    dst = ob[:, ng * 512:(ng + 1) * 512]
                    if on_scalar:
                        nc.scalar.copy(dst, ps[:])
                    else:
                        nc.vector.tensor_copy(dst, ps[:])
                nc.sync.dma_start(out=out[tt * 128:(tt + 1) * 128, :], in_=ob[:])

            def cp(tt):
                return lambda: emit_cproj_tt(tt)

            # qg0-qg2: attention then next group's projections (the proj block
            # is the PE work that overlaps the exp pipeline).  All c_proj moves
            # into qg3's exp-limited attention as bubble filler; tiles 10-11
            # (independent of qg3) land right after the last AV to cover the
            # final normalization chain, then qg3's own tiles finish.
            emit_proj(0)
            for qg in range(3):
                emit_attention(qg, 0)
                emit_attention(qg, 1)
                emit_proj(qg + 1)
            emit_attention(3, 0, fills=[cp(tt) for tt in range(0, 5)])
            emit_attention(3, 1, fills=[cp(tt) for tt in range(5, 8)], last=True)
            # tiles 8-11 don't depend on qg3: they run while the last
            # normalization chain completes and keep the PE warm
            for i, tt in enumerate(range(8, NTT)):
                emit_cproj_tt(tt, on_scalar=(i % 2 == 0))

    nc.compile()
    return nc


def kernel(x, w_attn, b_attn, w_proj, b_proj):
    global LAST_RESULTS
    x = np.asarray(x, dtype=np.float32)
    w_attn = np.asarray(w_attn, dtype=np.float32)
    b_attn = np.asarray(b_attn, dtype=np.float32)
    w_proj = np.asarray(w_proj, dtype=np.float32)
    b_proj = np.asarray(b_proj, dtype=np.float32)
    b, t, c = x.shape
    assert (b, t, c) == (2, T, C)

    if "nc" not in _CACHE:
        _CACHE["nc"] = _build()
    nc = _CACHE["nc"]

    trilm = np.triu(np.ones((128, 128), dtype=np.float32))  # [k, q]: valid iff k <= q
    in_maps = []
    for core in range(8):
        bi, g = divmod(core, 4)
        cs = FL * g  # column/row offset for this core's 4 heads
        wk = w_attn[:, C + cs:C + cs + FL]
        wq = w_attn[:, cs:cs + FL]
        wv = w_attn[:, 2 * C + cs:2 * C + cs + FL]
        bk = b_attn[C + cs:C + cs + FL]
        bq = b_attn[cs:cs + FL]
        bkq = np.stack([bk[0:128], bk[128:256], bq[0:128], bq[128:256]], axis=1)

        def img(w):  # [C, f] -> SBUF image [128, CK*f] (chunk ck at cols ck*f)
            f = w.shape[1]
            return np.ascontiguousarray(
                w.reshape(CK, 128, f).transpose(1, 0, 2).reshape(128, CK * f)).astype(BF)

        wp_l = w_proj[cs:cs + FL, :]
        in_maps.append({
            "x_img": np.ascontiguousarray(
                x[bi].T.reshape(CK, 128, NQG, 512).transpose(1, 2, 0, 3).reshape(128, CK * T)).astype(BF),
            "wk_img": img(wk),
            "wqv_img": img(np.concatenate([wq, wv], axis=1)),
            "wp_img": np.ascontiguousarray(
                wp_l.reshape(2, 128, C).transpose(1, 0, 2).reshape(128, 2 * C)).astype(BF),
            "bkq": np.ascontiguousarray(bkq),
            "bv": np.ascontiguousarray(b_attn[2 * C + cs:2 * C + cs + FL].reshape(1, FL)),
            "tril2": np.tile(trilm, (1, 2)).astype(BF),
            "vones": np.ones((128, NTT * NHL), dtype=BF),
        })

    res = run_bass_kernel_spmd(nc, in_maps, core_ids=list(range(8)))
    LAST_RESULTS = res
    # unshard: sum the 4 tensor-parallel partials of each batch element
    y = np.empty((2, T, C), dtype=np.float32)
    for bi in range(2):
        acc = res.results[4 * bi]["out"].astype(np.float32)
        for g in range(1, 4):
            acc = acc + res.results[4 * bi + g]["out"]
        y[bi] = acc + b_proj
    return y
